# revision 6
# baseline (speedup 1.0000x reference)
"""Trainium2 Bass kernel for nn_CFAggregator (GNN message passing) — v2.

Strategy (B-sharded data parallel over 8 cores, no collectives):
  - Host: pure index preprocessing. The two feature tables are packed into one
    [N, 256] fp16 table (agg|ff per node) so one indirect DMA per half fetches
    both self features. Per core, dedup'd edge weights (mask .set() semantics +
    1/cnt normalization), edges split into two signed-int16 index buckets and
    dest-sorted, densely packed into 128-slot chunks (chunk count = max over
    cores), block one-hot A maps gather slots -> dest columns (union windows).
  - Device: dma_gather (SWDGE) fetches per-edge fp16 feature rows; PE matmuls
    G_chunk^T @ A_chunk accumulate normalized neighbor sums in PSUM
    (feature-major). Everything downstream runs fp16 feature-major on a single
    activation table set (exp+ln; 1/sqrt via exp(-0.5 ln)). Self-feature stats
    fold into the neighbor stats matmuls via 0-stride broadcast rhs APs.
All feature-table traffic happens on-device; the host only touches index
tensors and small weights.
"""

import numpy as np
import ml_dtypes

import concourse.bass as bass
import concourse.bacc as bacc
import concourse.tile as tile
from concourse import mybir
from concourse.bass_utils import run_bass_kernel_spmd
from concourse.masks import make_identity

F32 = mybir.dt.float32
F16 = mybir.dt.float16
I32 = mybir.dt.int32
I16 = mybir.dt.int16
AF = mybir.ActivationFunctionType
OP = mybir.AluOpType
NPF16 = np.float16

# problem dims (hardcoded per contract)
B, MC, U, N, DIN, DOUT, E = 2048, 4, 20000, 100000, 128, 128, 65536
RES_RATE = 0.9
NCORES = 8
BC = B // NCORES          # 256 nodes per core
DEST = BC * MC            # 1024 destination columns per core
P = 128
FW = 2 * DIN              # 256: fused row width (agg|ff)

# int16 bucket bases over node space [0, 100000)
BUCKET_BASES = (32768, 82768)
BUCKET_LO = (0, 65536)
PIECE = 8  # chunks per dma_gather (1024 idx = hard ucode packet limit)

# consts tile slots (each [128, 128] fp16): 4 DMA'd weights + 5 synthesized
(S_WK, S_WQ, S_WVA, S_WVF) = range(4)
(S_ONES, S_ONESC, S_MULO, S_MUHI, S_ID) = range(5)
NSLOT = 4
ENG = dict(nraw='act', sqn='act', actncp='dve', mix='dve', rp='dve')


# --------------------------------------------------------------------------
# host-side preprocessing (index math only)
# --------------------------------------------------------------------------

def _wrap_idx16(idx_flat):
    """int16 index list -> [128, ceil(n/16)] wrapped in 16 partitions, x8."""
    n = len(idx_flat)
    cols = (n + 15) // 16
    pad = np.zeros(cols * 16, np.int16)
    pad[:n] = idx_flat.astype(np.int16)
    w16 = pad.reshape(cols, 16).T
    return np.ascontiguousarray(np.tile(w16, (8, 1)))


def preprocess(inputs):
    """Build per-core gather/index/one-hot structures. Returns (plan, percore)."""
    nodes = np.asarray(inputs["nodes"]).astype(np.int64)
    unique_ids = np.asarray(inputs["unique_ids"]).astype(np.int64)
    row_idx = np.asarray(inputs["row_idx"]).astype(np.int64)
    layer_idx = np.asarray(inputs["layer_idx"]).astype(np.int64)
    col_idx = np.asarray(inputs["col_idx"]).astype(np.int64)

    eff = unique_ids[col_idx]                       # table row per edge
    # dedup (b, layer, col) triples: .set() counts duplicates once
    key = (row_idx * MC + layer_idx) * U + col_idx
    uniq_keys, first_pos = np.unique(key, return_index=True)
    keep = np.zeros(E, bool)
    keep[first_pos] = True
    grp_of_uniq = uniq_keys // U
    cnt = np.bincount(grp_of_uniq, minlength=B * MC)
    grp = row_idx * MC + layer_idx
    w = np.where(keep, 1.0 / np.maximum(cnt[grp], 1), 0.0).astype(np.float32)
    dest_all = (row_idx % BC) * MC + layer_idx      # core-local dest column

    # per (core, bucket): dest-sorted edge lists
    core_lists = []
    for c in range(NCORES):
        sel = (row_idx >= c * BC) & (row_idx < (c + 1) * BC)
        e_eff, e_dest, e_w = eff[sel], dest_all[sel], w[sel]
        per_bucket = []
        for r in range(2):
            bsel = (e_eff >= BUCKET_LO[r]) & (e_eff < (BUCKET_LO[1] if r == 0 else N))
            order = np.argsort(e_dest[bsel], kind="stable")
            per_bucket.append((
                (e_eff[bsel][order] - BUCKET_BASES[r]).astype(np.int32),
                e_dest[bsel][order].astype(np.int32),
                e_w[bsel][order],
            ))
        core_lists.append(per_bucket)

    # dense chunking: chunk = 128 consecutive dest-sorted edges; chunk count =
    # max over cores (shared compiled module), shorter cores pad (idx 0, w 0).
    nchk = []
    for r in range(2):
        mx = max(len(core_lists[c][r][0]) for c in range(NCORES))
        nchk.append((mx + 127) // 128)

    core_streams = []   # [core][bucket] -> (idx, dest, w) padded to nchk*128
    for c in range(NCORES):
        per_bucket = []
        for r in range(2):
            idx_rel, dests, ws = core_lists[c][r]
            cap = nchk[r] * 128
            s_idx = np.zeros(cap, np.int32)
            s_dst = np.full(cap, -1, np.int32)
            s_w = np.zeros(cap, np.float32)
            n = len(idx_rel)
            # spread edges evenly over the chunk grid (quantile alignment
            # across cores keeps per-chunk dest unions tight)
            bnd = np.round(np.arange(nchk[r] + 1) * n / nchk[r]).astype(np.int64)
            for k in range(nchk[r]):
                e0, e1 = bnd[k], bnd[k + 1]
                s_idx[k * 128:k * 128 + e1 - e0] = idx_rel[e0:e1]
                s_dst[k * 128:k * 128 + e1 - e0] = dests[e0:e1]
                s_w[k * 128:k * 128 + e1 - e0] = ws[e0:e1]
            per_bucket.append((s_idx, s_dst, s_w))
        core_streams.append(per_bucket)

    # gather pieces: runs of <= PIECE chunks
    pieces = []
    for r in range(2):
        bounds = list(range(0, nchk[r], PIECE)) + [nchk[r]]
        pieces.append([(bounds[i], bounds[i + 1]) for i in range(len(bounds) - 1)])

    # each gather piece must END on idx >= 0 (ucode pops trailing negatives):
    # swap a nonneg idx (pads are 0) into the last slot within the final chunk
    for c in range(NCORES):
        for r in range(2):
            s_idx, s_dst, s_w = core_streams[c][r]
            for (k0, k1) in pieces[r]:
                last = k1 * 128 - 1
                if s_idx[last] >= 0:
                    continue
                ch0 = (k1 - 1) * 128
                cand = np.nonzero(s_idx[ch0:last] >= 0)[0]
                assert len(cand), "whole final chunk of a piece is negative"
                j = ch0 + cand[0]
                for arr in (s_idx, s_dst, s_w):
                    arr[j], arr[last] = arr[last], arr[j]

    # per-chunk dest spans = union over cores of real dests
    spans = []          # (r, k, lo, hi)
    for r in range(2):
        for k in range(nchk[r]):
            lo, hi = DEST, 0
            for c in range(NCORES):
                d = core_streams[c][r][1][k * 128:(k + 1) * 128]
                d = d[d >= 0]
                if len(d):
                    lo = min(lo, int(d.min()))
                    hi = max(hi, int(d.max()) + 1)
            if hi <= lo:
                lo, hi = -1, -1
            spans.append((r, k, lo, hi))

    # segments: split spans at 256-column quarter-tile boundaries
    segs = []
    acol = 0
    for (r, k, lo, hi) in spans:
        if lo < 0:
            continue
        for t in range(4):
            b0, b1 = t * 256, (t + 1) * 256
            s0, s1 = max(lo, b0), min(hi, b1)
            if s1 > s0:
                segs.append(dict(bucket=r, chunk=k, tile=t, lo=s0, hi=s1,
                                 acol=acol + (s0 - lo)))
        acol += hi - lo
    aw = max(acol, 1)

    plan = dict(nchk=tuple(nchk), segs=segs, aw=aw,
                pieces=(tuple(pieces[0]), tuple(pieces[1])))

    percore = []
    span_acol = {}
    ac = 0
    for (r, k, lo, hi) in spans:
        span_acol[(r, k)] = (ac, lo)
        if lo >= 0:
            ac += hi - lo
    for c in range(NCORES):
        amat = np.zeros((P, aw), NPF16)
        widx = []
        for r in range(2):
            s_idx, s_dst, s_w = core_streams[c][r]
            assert s_idx.max(initial=0) <= 32767 and s_idx.min(initial=0) >= -32768
            widx.append(_wrap_idx16(s_idx))
            for k in range(nchk[r]):
                a0, lo = span_acol[(r, k)]
                if lo < 0:
                    continue
                sl = slice(k * 128, (k + 1) * 128)
                real = s_dst[sl] >= 0
                pp = np.nonzero(real)[0]
                amat[pp, a0 + s_dst[sl][pp] - lo] = s_w[sl][pp].astype(NPF16)
        sidx = np.zeros((P, 2), np.int32)
        sidx[:, 0] = nodes[c * BC: c * BC + 128]
        sidx[:, 1] = nodes[c * BC + 128: (c + 1) * BC]
        percore.append(dict(amat=amat, widx0=widx[0], widx1=widx[1],
                            widx=np.concatenate([widx[0], widx[1]], axis=1),
                            sidx=sidx))

    return plan, percore


def make_consts(inputs):
    """([128, 4*128] fp16 weights, [128, 2] f32 mu) shared across cores."""
    c = np.zeros((P, NSLOT * 128), NPF16)
    c[:, S_WK * 128:(S_WK + 1) * 128] = np.asarray(inputs["Wk"], np.float32)
    c[:, S_WQ * 128:(S_WQ + 1) * 128] = np.asarray(inputs["Wq"], np.float32)
    c[:, S_WVA * 128:(S_WVA + 1) * 128] = np.asarray(inputs["Wv_agg"], np.float32)
    c[:, S_WVF * 128:(S_WVF + 1) * 128] = np.asarray(inputs["Wv_ff"], np.float32)
    mu = np.asarray(inputs["mu_w"]).astype(np.float32).reshape(2, DOUT).T
    wva = np.asarray(inputs["Wv_agg"], np.float32)
    folded = np.stack([wva @ mu[:, 1], wva @ mu[:, 0]], axis=1)  # [w_num | w_smu]
    return c, np.ascontiguousarray(folded)


def make_big_table(inputs):
    """([N, 128] fp16 agg-only, [N, 256] fp16 agg|ff)."""
    agg16 = np.asarray(inputs["agg_table"], np.float32).astype(NPF16)
    big = np.empty((N, FW), NPF16)
    big[:, 0:DIN] = agg16
    big[:, DIN:FW] = np.asarray(inputs["ff_table"], np.float32)
    return agg16, big


# --------------------------------------------------------------------------
# device module
# --------------------------------------------------------------------------

def build_module(plan):
    nchk = plan["nchk"]
    aw = plan["aw"]
    segs = plan["segs"]

    nc = bacc.Bacc("TRN2", target_bir_lowering=False, debug=False,
                   num_devices=NCORES, num_swdge_queues=4)

    big_t = nc.dram_tensor("big", [N, FW], F16, kind="ExternalInput")
    consts = nc.dram_tensor("consts", [P, NSLOT * 128], F16, kind="ExternalInput")
    mu_d = nc.dram_tensor("mu", [P, 2], F32, kind="ExternalInput")
    amat = nc.dram_tensor("amat", [P, aw], F16, kind="ExternalInput")
    widx0 = nc.dram_tensor("widx0", [P, nchk[0] * 8], I16, kind="ExternalInput")
    widx1 = nc.dram_tensor("widx1", [P, nchk[1] * 8], I16, kind="ExternalInput")
    sidx = nc.dram_tensor("sidx", [P, 2], I32, kind="ExternalInput")
    out_t = nc.dram_tensor("out", [2, P, BC], F16, kind="ExternalOutput")

    # piece order on the gpsimd queue: self halves early (they gate the
    # highway front), then buckets interleaved so tile-0 chunks land first
    np0, np1 = len(plan["pieces"][0]), len(plan["pieces"][1])
    ei = []
    for i in range(max(np0, np1)):
        if i < np0:
            ei.append((0, *plan["pieces"][0][i]))
        if i < np1:
            ei.append((1, *plan["pieces"][1][i]))
    order = [('edge', *ei[0]), ('self', 0), ('self', 1)]
    order += [('edge', *e) for e in ei[1:]]

    # last chunk index per (tile) for stop flags
    last_per_tile = {}
    for i, s in enumerate(segs):
        last_per_tile[s["tile"]] = i

    with tile.TileContext(nc) as tc:
        with (
            nc.allow_low_precision(reason="fp16 pipeline validated vs 2e-2 tol"),
            tc.tile_pool(name="sb", bufs=1) as sb,
            tc.tile_pool(name="psA", bufs=4, space="PSUM") as psA,
            tc.tile_pool(name="ps", bufs=2, space="PSUM") as ps,
        ):
            def slot(k):
                return c_sb[:, k * 128:(k + 1) * 128]

            def syn(k):
                return syn_sb[:, k * 128:(k + 1) * 128]

            # ---- input DMAs (tiny index tensors first; A on the ACT ring)
            c_sb = sb.tile([P, NSLOT * 128], F16, tag="c_sb")
            a_sb = sb.tile([P, aw], F16, tag="a_sb")
            mu_sb = sb.tile([P, 2], F32, tag="mu_sb")
            w0_sb = sb.tile([P, nchk[0] * 8], I16, tag="w0_sb")
            w1_sb = sb.tile([P, nchk[1] * 8], I16, tag="w1_sb")
            si_sb = sb.tile([P, 2], I32, tag="si_sb")
            nc.sync.dma_start(out=w0_sb[:], in_=widx0[:, :])
            nc.sync.dma_start(out=w1_sb[:], in_=widx1[:, :])
            nc.sync.dma_start(out=si_sb[:], in_=sidx[:, :])
            nc.sync.dma_start(out=c_sb[:], in_=consts[:, :])
            nc.sync.dma_start(out=mu_sb[:], in_=mu_d[:, :])
            nc.scalar.dma_start(out=a_sb[:], in_=amat[:, :])

            # prime the single ACT table set (exp+ln) at t=0
            warm = sb.tile([P, 1], F32, tag="warm")
            nc.vector.memset(warm[:], 1.0)
            warm2 = sb.tile([P, 2], F32, tag="warm2")
            nc.scalar.activation(warm2[:, 0:1], warm[:], AF.Ln)
            nc.scalar.activation(warm2[:, 1:2], warm[:], AF.Exp)

            # synthesized constants (fp16)
            syn_sb = sb.tile([P, 5 * 128], F16, tag="syn_sb")
            nc.vector.memset(syn_sb[:, S_ONES * 128:(S_ONES + 1) * 128], 1.0)
            nc.vector.memset(syn_sb[:, S_ONESC * 128:(S_ONESC + 1) * 128], 1.0 / DOUT)
            # mu_d col0 = w_num (neighbor vector), col1 = w_smu (self vector)
            nc.vector.tensor_copy(syn_sb[:, S_MUHI * 128:(S_MUHI + 1) * 128],
                                  mu_sb[:, 0:1].to_broadcast((P, 128)))
            nc.vector.tensor_copy(syn_sb[:, S_MULO * 128:(S_MULO + 1) * 128],
                                  mu_sb[:, 1:2].to_broadcast((P, 128)))
            make_identity(nc, syn_sb[:, S_ID * 128:(S_ID + 1) * 128])

            # aggregation psums zeroed early (before any dependent DVE work
            # queues ahead of them in the in-order DVE stream)
            pagg = [psA.tile([P, 512], F32, tag="pagg", name=f"pagg{i}")
                    for i in range(2)]
            nc.vector.memset(pagg[0][:], 0.0)
            nc.vector.memset(pagg[1][:], 0.0)

            # ---- dummy gather: loads the mlp ucode library at t=0
            dum_i = sb.tile([P, 8], I16, tag="dum_i")
            nc.gpsimd.memset(dum_i[:], 0)
            dum_o = sb.tile([P, 1, 128], F16, tag="dum_o")
            nc.gpsimd.dma_gather(dum_o[:], big_t[:, 0:128], dum_i[:], 128, 128, 128,
                                 elem_step=FW, queue_num=1)

            # ---- gathers (edge pieces + self indirects) in `order`
            g0 = sb.tile([P, nchk[0], 128], F16, tag="g0")
            g1 = sb.tile([P, nchk[1], 128], F16, tag="g1")
            sr = sb.tile([P, 2, FW], F16, tag="sr")     # [part=b%128, h, agg|ff]
            gtiles = (g0, g1)
            tiles_w = ((g0, w0_sb), (g1, w1_sb))
            gq = 0
            for item in order:
                if item[0] == 'self':
                    h = item[1]
                    nc.gpsimd.indirect_dma_start(
                        out=sr[:, h, :], out_offset=None,
                        in_=big_t[:, :],
                        in_offset=bass.IndirectOffsetOnAxis(ap=si_sb[:, h:h + 1], axis=0))
                else:
                    _, r, k0, k1 = item
                    gt, wt = tiles_w[r]
                    nc.gpsimd.dma_gather(
                        gt[:, k0:k1, :], big_t[BUCKET_BASES[r]:, 0:128],
                        wt[:, k0 * 8:k1 * 8],
                        (k1 - k0) * 128, (k1 - k0) * 128, 128,
                        elem_step=FW, queue_num=gq % 4)
                    gq += 1

            # ---- pair transposes: pair_T [128 feat, 512] =
            #      [aggT_h0 | aggT_h1 | ffT_h0 | ffT_h1]
            pair_T = sb.tile([P, 512], F16, tag="pair_T")
            for h in range(2):
                tpa = ps.tile([P, 128], F16, tag="ps_rot", name=f"tpa{h}")
                nc.tensor.transpose(tpa[:], sr[:, h, 0:128], syn(S_ID))
                nc.scalar.copy(pair_T[:, h * 128:(h + 1) * 128], tpa[:])
                tpf = ps.tile([P, 128], F16, tag="ps_rot", name=f"tpf{h}")
                nc.tensor.transpose(tpf[:], sr[:, h, 128:256], syn(S_ID))
                nc.scalar.copy(pair_T[:, 256 + h * 128: 256 + (h + 1) * 128], tpf[:])

            # ---- early dense from pair_T: K/Q (both branches), vf, self acts
            kt_ps = ps.tile([P, 512], F32, tag="ps_kq", name="kt_ps", bufs=1)
            nc.tensor.matmul(out=kt_ps[:], lhsT=slot(S_WK), rhs=pair_T[:],
                             start=True, stop=True)
            kt = sb.tile([P, 512], F16, tag="kt")
            nc.scalar.copy(kt[:], kt_ps[:])
            qt_ps = ps.tile([P, 512], F32, tag="ps_kq", name="qt_ps", bufs=1)
            nc.tensor.matmul(out=qt_ps[:], lhsT=slot(S_WQ), rhs=pair_T[:],
                             start=True, stop=True)
            qt = sb.tile([P, 512], F16, tag="qt")
            nc.scalar.copy(qt[:], qt_ps[:])
            vfs_ps = ps.tile([P, 512], F32, tag="ps_big", name="vfs_ps", bufs=1)
            nc.tensor.matmul(out=vfs_ps[:, 0:256], lhsT=slot(S_WVF),
                             rhs=pair_T[:, 256:512], start=True, stop=True,
                             skip_group_check=True)
            nc.tensor.matmul(out=vfs_ps[:, 256:512], lhsT=slot(S_WVA),
                             rhs=pair_T[:, 0:256], start=True, stop=True,
                             skip_group_check=True)
            vf = sb.tile([P, 256], F16, tag="vf")
            nc.scalar.copy(vf[:], vfs_ps[:, 0:256])
            acts = sb.tile([P, 256], F16, tag="acts")   # self branch acts
            nc.vector.tensor_copy(acts[:], vfs_ps[:, 256:512])
            sqs = sb.tile([P, 256], F16, tag="sqs")
            nc.vector.tensor_mul(sqs[:], acts[:], acts[:])
            self_half = sb.tile([P, 256], F16, tag="self_half")
            nc.scalar.mul(self_half[:], acts[:], 0.5)
            vf01 = sb.tile([P, 256], F16, tag="vf01")
            nc.scalar.mul(vf01[:], vf[:], 1.0 - RES_RATE)

            # highway front: dif_i = colsum(K_i * (Q_agg - Q_ff)) / DOUT
            qd = sb.tile([P, 256], F16, tag="qd")
            nc.vector.tensor_sub(qd[:], qt[:, 0:256], qt[:, 256:512])
            pd = sb.tile([P, 512], F16, tag="pd")
            nc.vector.tensor_mul(pd[:, 0:256], kt[:, 0:256], qd[:])
            nc.vector.tensor_mul(pd[:, 256:512], kt[:, 256:512], qd[:])
            dif_ps = ps.tile([P, 512], F32, tag="ps_big", name="dif_ps", bufs=1)
            nc.tensor.matmul(out=dif_ps[:], lhsT=syn(S_ONESC), rhs=pd[:],
                             start=True, stop=True)
            eneg = sb.tile([P, 512], F16, tag="eneg")
            nc.scalar.activation(eneg[:], dif_ps[:], AF.Exp, scale=-1.0)
            wden = sb.tile([P, 512], F16, tag="wden")
            nc.vector.tensor_scalar_add(wden[:], eneg[:], 1.0)
            wgt = sb.tile([P, 512], F16, tag="wgt")      # [waa 256 | wfa 256]
            nc.vector.reciprocal(wgt[:], wden[:])

            # ---- per-half chains
            nraw = sb.tile([P, 1024], F16, tag="nraw")
            actn = sb.tile([P, 1024], F16, tag="actn")
            sqn = sb.tile([P, 1024], F16, tag="sqn")
            logit = sb.tile([P, 1024], F16, tag="logit")
            esm = sb.tile([P, 1024], F16, tag="esm")
            tmul = sb.tile([P, 1024], F16, tag="tmul")
            tsum = sb.tile([P, 256], F32, tag="tsum")
            ssum = sb.tile([P, 256], F32, tag="ssum")
            rsum = sb.tile([P, 256], F32, tag="rsum")
            nsum = sb.tile([P, 256], F16, tag="nsum")
            vmid = sb.tile([P, 256], F16, tag="vmid")
            lden = sb.tile([P, 1024], F16, tag="lden")
            rden = sb.tile([P, 1024], F16, tag="rden")
            pre = sb.tile([P, 512], F16, tag="pre")     # [agg_h0|agg_h1|ff_h0|ff_h1]
            out_sb = sb.tile([P, 512], F16, tag="out_sb")

            rep4 = lambda apx: apx[:, :, None].to_broadcast((P, 128, MC))

            for h in range(2):
                hs = slice(h * 512, (h + 1) * 512)
                for i, s in enumerate(segs):
                    if s["tile"] != h:
                        continue
                    nc.tensor.matmul(
                        out=pagg[h][:, s["lo"] - h * 512: s["hi"] - h * 512],
                        lhsT=gtiles[s["bucket"]][:, s["chunk"], :],
                        rhs=a_sb[:, s["acol"]: s["acol"] + s["hi"] - s["lo"]],
                        start=False, stop=(last_per_tile[h] == i),
                        skip_group_check=True)
                # PSUM -> SBUF fp16 copy, split ACT || DVE
                nc.scalar.copy(nraw[:, h * 512:h * 512 + 256], pagg[h][:, 0:256])
                nc.vector.tensor_copy(nraw[:, h * 512 + 256:(h + 1) * 512],
                                      pagg[h][:, 256:512])
                actn_ps = psA.tile([P, 512], F32, tag="pagg", name=f"actn_ps{h}")
                nc.tensor.matmul(out=actn_ps[:], lhsT=slot(S_WVA), rhs=nraw[:, hs],
                                 start=True, stop=True)
                nc.scalar.copy(actn[:, h * 512:h * 512 + 256], actn_ps[:, 0:256])
                nc.vector.tensor_copy(actn[:, h * 512 + 256:(h + 1) * 512],
                                      actn_ps[:, 256:512])
                nc.vector.tensor_mul(sqn[:, hs], actn[:, hs], actn[:, hs])
                # stats matmuls with self fold-in (0-stride broadcast rhs):
                # den2 = ones@sqn + ones@rep4(sqs_h); num = muhi@actn + mulo@rep4(acts_h)
                den2_ps = psA.tile([P, 512], F32, tag="pagg", name=f"den2_ps{h}")
                nc.tensor.matmul(out=den2_ps[:], lhsT=syn(S_ONES), rhs=sqn[:, hs],
                                 start=True, stop=False, skip_group_check=True)
                nc.tensor.matmul(out=den2_ps[:].rearrange("p (b m) -> p b m", m=MC),
                                 lhsT=syn(S_ONES),
                                 rhs=rep4(sqs[:, h * 128:(h + 1) * 128]),
                                 start=False, stop=True, skip_group_check=True)
                num_ps = psA.tile([P, 512], F32, tag="pagg", name=f"num_ps{h}")
                nc.tensor.matmul(out=num_ps[:], lhsT=syn(S_MUHI), rhs=actn[:, hs],
                                 start=True, stop=False, skip_group_check=True)
                nc.tensor.matmul(out=num_ps[:].rearrange("p (b m) -> p b m", m=MC),
                                 lhsT=syn(S_MULO),
                                 rhs=rep4(acts[:, h * 128:(h + 1) * 128]),
                                 start=False, stop=True, skip_group_check=True)
                # rden = exp(-0.5 ln(den2)); logit = num * rden
                nc.scalar.activation(lden[:, hs], den2_ps[:], AF.Ln)
                nc.scalar.activation(rden[:, hs], lden[:, hs], AF.Exp, scale=-0.5)
                nc.vector.tensor_mul(logit[:, hs], num_ps[:], rden[:, hs])
                nc.scalar.activation(esm[:, hs], logit[:, hs], AF.Exp)
                nc.vector.tensor_mul(tmul[:, hs], esm[:, hs], actn[:, hs])
                ts_h = slice(h * 128, (h + 1) * 128)
                nc.vector.reduce_sum(
                    out=tsum[:, ts_h],
                    in_=tmul[:, hs].rearrange("p (b m) -> p b m", m=MC),
                    axis=mybir.AxisListType.X)
                nc.vector.reduce_sum(
                    out=ssum[:, ts_h],
                    in_=esm[:, hs].rearrange("p (b m) -> p b m", m=MC),
                    axis=mybir.AxisListType.X)
                nc.vector.reciprocal(rsum[:, ts_h], ssum[:, ts_h])
                nc.vector.tensor_mul(nsum[:, ts_h], tsum[:, ts_h], rsum[:, ts_h])
                nc.vector.scalar_tensor_tensor(
                    out=vmid[:, ts_h], in0=nsum[:, ts_h], scalar=0.5,
                    in1=self_half[:, ts_h], op0=OP.mult, op1=OP.add)

                # residual mix + ELU per half:
                #   pre_agg = 0.9*vmid + 0.1*vf + 0.1*waa*dd
                #   pre_ff  = vf + 0.1*wfa*dd
                dd_h = sb.tile([P, 128], F16, tag=f"dd{h}", name=f"dd{h}")
                nc.vector.tensor_sub(dd_h[:], vmid[:, ts_h], vf[:, ts_h])
                base_h = sb.tile([P, 128], F16, tag=f"base{h}", name=f"base{h}")
                nc.vector.scalar_tensor_tensor(
                    out=base_h[:], in0=vmid[:, ts_h], scalar=RES_RATE,
                    in1=vf01[:, ts_h], op0=OP.mult, op1=OP.add)
                for o, b9 in enumerate((base_h[:], vf[:, ts_h])):
                    nw = sb.tile([P, 128], F16, tag=f"nw{h}{o}", name=f"nw{h}{o}")
                    nc.vector.tensor_mul(nw[:], wgt[:, o * 256 + h * 128:
                                                    o * 256 + (h + 1) * 128], dd_h[:])
                    nc.vector.scalar_tensor_tensor(
                        out=pre[:, o * 256 + h * 128: o * 256 + (h + 1) * 128],
                        in0=nw[:], scalar=1.0 - RES_RATE, in1=b9,
                        op0=OP.mult, op1=OP.add)
                # ELU: relu(x) + min(exp(x), 1) - 1
                for o in range(2):
                    sl = slice(o * 256 + h * 128, o * 256 + (h + 1) * 128)
                    ep = sb.tile([P, 128], F16, tag=f"ep{h}{o}", name=f"ep{h}{o}")
                    nc.scalar.activation(ep[:], pre[:, sl], AF.Exp)
                    rp = sb.tile([P, 128], F16, tag=f"rp{h}{o}", name=f"rp{h}{o}")
                    nc.vector.tensor_scalar_max(rp[:], pre[:, sl], 0.0)
                    em = sb.tile([P, 128], F16, tag=f"em{h}{o}", name=f"em{h}{o}")
                    nc.vector.tensor_scalar(out=em[:], in0=ep[:], scalar1=1.0,
                                            scalar2=-1.0, op0=OP.min, op1=OP.add)
                    nc.vector.tensor_add(out_sb[:, sl], em[:], rp[:])
                # per-half output DMA: out[c, :, h*128:(h+1)*128]
                nc.sync.dma_start(
                    out=out_t[:, :, h * 128:(h + 1) * 128].rearrange("c d b -> d c b"),
                    in_=out_sb[:].rearrange("p (c b) -> p c b", b=256)[:, :, h * 128:(h + 1) * 128])

    nc.compile()
    return nc


# --------------------------------------------------------------------------
# numpy simulation of the device pipeline (validates preprocessing + math)
# --------------------------------------------------------------------------

def numpy_simulate(inputs, plan, percore):
    big = make_big_table(inputs)[1].astype(np.float32)
    cmat, mu2 = make_consts(inputs)
    cmat = cmat.astype(np.float32)
    outs_a, outs_f = [], []
    for c in range(NCORES):
        pc = percore[c]
        def unwrap(widx, nchunks):
            w16 = widx[:16, :]
            return w16.T.reshape(-1).astype(np.int32)[: nchunks * 128]
        g = []
        for r, widx in enumerate((pc["widx0"], pc["widx1"])):
            idx = unwrap(widx, plan["nchk"][r]) + BUCKET_BASES[r]
            g.append(big[idx, 0:128].reshape(plan["nchk"][r], 128, 128).transpose(1, 0, 2))
        srn = big[pc["sidx"].T.reshape(-1)]          # [256, 256] node-major
        pair_T = np.concatenate([srn[:, 0:128].T, srn[:, 128:256].T], axis=1)
        pagg = np.zeros((4, P, 256), np.float32)
        for s in plan["segs"]:
            G = g[s["bucket"]][:, s["chunk"], :]
            A = pc["amat"].astype(np.float32)[:, s["acol"]: s["acol"] + s["hi"] - s["lo"]]
            pagg[s["tile"]][:, s["lo"] - s["tile"] * 256: s["hi"] - s["tile"] * 256] += G.T @ A
        neigh_rawT = np.concatenate(list(pagg), axis=1)
        Wva = cmat[:, S_WVA * 128:(S_WVA + 1) * 128]
        Wvf = cmat[:, S_WVF * 128:(S_WVF + 1) * 128]
        Wk = cmat[:, S_WK * 128:(S_WK + 1) * 128]
        Wq = cmat[:, S_WQ * 128:(S_WQ + 1) * 128]
        actn = Wva.T @ neigh_rawT                     # [128, 1024]
        acts = Wva.T @ pair_T[:, 0:256]               # [128, 256] self
        vf = Wvf.T @ pair_T[:, 256:512]
        kt = Wk.T @ pair_T
        qt = Wq.T @ pair_T
        n2 = (actn * actn).sum(0)
        s2 = (acts * acts).sum(0)
        w_num, w_smu = mu2[:, 0:1], mu2[:, 1:2]
        nmu = (w_num * neigh_rawT).sum(0)
        smu = (w_smu * pair_T[:, 0:256]).sum(0)
        den2 = n2 + np.repeat(s2, MC)
        numv = nmu + np.repeat(smu, MC)
        logit = numv / np.sqrt(den2)
        e = np.exp(logit).reshape(BC, MC)
        coef = e / e.sum(1, keepdims=True)
        neighT = actn.reshape(P, BC, MC)
        nsum = (neighT * coef[None]).sum(-1)
        vmid = 0.5 * (acts + nsum)
        saa = (kt[:, 0:256] * qt[:, 0:256]).sum(0) / DOUT
        saf = (kt[:, 0:256] * qt[:, 256:512]).sum(0) / DOUT
        sfa = (kt[:, 256:512] * qt[:, 0:256]).sum(0) / DOUT
        sff = (kt[:, 256:512] * qt[:, 256:512]).sum(0) / DOUT
        waa = 1.0 / (1.0 + np.exp(-(saa - saf)))
        wfa = 1.0 / (1.0 + np.exp(-(sfa - sff)))
        dd = vmid - vf
        new0 = vf + waa[None] * dd
        new1 = vf + wfa[None] * dd
        pre0 = RES_RATE * vmid + (1 - RES_RATE) * new0
        pre1 = RES_RATE * vf + (1 - RES_RATE) * new1
        elu = lambda x: np.where(x > 0, x, np.exp(np.minimum(x, 0)) - 1)
        outs_a.append(elu(pre0).T)
        outs_f.append(elu(pre1).T)
    return np.concatenate(outs_a, 0), np.concatenate(outs_f, 0)


# --------------------------------------------------------------------------
# public entry point
# --------------------------------------------------------------------------

_module_cache = {}
_last_results = None


def _plan_signature(plan):
    return (plan["nchk"], plan["aw"], plan["pieces"],
            tuple((s["bucket"], s["chunk"], s["tile"], s["lo"], s["hi"], s["acol"])
                  for s in plan["segs"]))


def kernel(**inputs):
    plan, percore = preprocess(inputs)
    sig = _plan_signature(plan)
    if sig not in _module_cache:
        _module_cache[sig] = build_module(plan)
    nc = _module_cache[sig]

    cmat, mu2 = make_consts(inputs)
    agg16, big = make_big_table(inputs)
    in_maps = []
    for c in range(NCORES):
        pc = percore[c]
        in_maps.append({
            "agg16": agg16,
            "big": big,
            "consts": cmat,
            "mu": mu2,
            "amat": pc["amat"],
            "widx": pc["widx"],
            "sidx": pc["sidx"],
        })
    res = run_bass_kernel_spmd(nc, in_maps, core_ids=list(range(NCORES)))
    global _last_results
    _last_results = res
    agg_out = np.concatenate(
        [res.results[c]["out"][0].astype(np.float32).T for c in range(NCORES)], axis=0)
    ff_out = np.concatenate(
        [res.results[c]["out"][1].astype(np.float32).T for c in range(NCORES)], axis=0)
    return agg_out, ff_out


# revision 7
# speedup vs baseline: 1.0072x; 1.0072x over previous
"""Trainium2 Bass kernel for nn_CFAggregator (GNN message passing) — v2.

Strategy (B-sharded data parallel over 8 cores, no collectives):
  - Host: pure index preprocessing. The two feature tables are packed into one
    [N, 256] fp16 table (agg|ff per node) so one indirect DMA per half fetches
    both self features. Per core, dedup'd edge weights (mask .set() semantics +
    1/cnt normalization), edges split into two signed-int16 index buckets and
    dest-sorted, densely packed into 128-slot chunks (chunk count = max over
    cores), block one-hot A maps gather slots -> dest columns (union windows).
  - Device: dma_gather (SWDGE) fetches per-edge fp16 feature rows; PE matmuls
    G_chunk^T @ A_chunk accumulate normalized neighbor sums in PSUM
    (feature-major). Everything downstream runs fp16 feature-major on a single
    activation table set (exp+ln; 1/sqrt via exp(-0.5 ln)). Self-feature stats
    fold into the neighbor stats matmuls via 0-stride broadcast rhs APs.
All feature-table traffic happens on-device; the host only touches index
tensors and small weights.
"""

import numpy as np
import ml_dtypes

import concourse.bass as bass
import concourse.bacc as bacc
import concourse.tile as tile
from concourse import mybir
from concourse.bass_utils import run_bass_kernel_spmd
from concourse.masks import make_identity

F32 = mybir.dt.float32
F16 = mybir.dt.float16
I32 = mybir.dt.int32
I16 = mybir.dt.int16
AF = mybir.ActivationFunctionType
OP = mybir.AluOpType
NPF16 = np.float16

# problem dims (hardcoded per contract)
B, MC, U, N, DIN, DOUT, E = 2048, 4, 20000, 100000, 128, 128, 65536
RES_RATE = 0.9
NCORES = 8
BC = B // NCORES          # 256 nodes per core
DEST = BC * MC            # 1024 destination columns per core
P = 128
FW = 2 * DIN              # 256: fused row width (agg|ff)

# int16 bucket bases over node space [0, 100000)
BUCKET_BASES = (32768, 82768)
BUCKET_LO = (0, 65536)
PIECE = 8  # chunks per dma_gather (1024 idx = hard ucode packet limit)

# consts tile slots (each [128, 128] fp16): 4 DMA'd weights + 5 synthesized
(S_WK, S_WQ, S_WVA, S_WVF) = range(4)
(S_ONES, S_ONESC, S_MULO, S_MUHI, S_ID) = range(5)
NSLOT = 4
ENG = dict(nraw='act', sqn='act', actncp='dve', mix='dve', rp='dve', soff=1.2)


# --------------------------------------------------------------------------
# host-side preprocessing (index math only)
# --------------------------------------------------------------------------

def _wrap_idx16(idx_flat):
    """int16 index list -> [128, ceil(n/16)] wrapped in 16 partitions, x8."""
    n = len(idx_flat)
    cols = (n + 15) // 16
    pad = np.zeros(cols * 16, np.int16)
    pad[:n] = idx_flat.astype(np.int16)
    w16 = pad.reshape(cols, 16).T
    return np.ascontiguousarray(np.tile(w16, (8, 1)))


def preprocess(inputs):
    """Build per-core gather/index/one-hot structures. Returns (plan, percore)."""
    nodes = np.asarray(inputs["nodes"]).astype(np.int64)
    unique_ids = np.asarray(inputs["unique_ids"]).astype(np.int64)
    row_idx = np.asarray(inputs["row_idx"]).astype(np.int64)
    layer_idx = np.asarray(inputs["layer_idx"]).astype(np.int64)
    col_idx = np.asarray(inputs["col_idx"]).astype(np.int64)

    eff = unique_ids[col_idx]                       # table row per edge
    # dedup (b, layer, col) triples: .set() counts duplicates once
    key = (row_idx * MC + layer_idx) * U + col_idx
    uniq_keys, first_pos = np.unique(key, return_index=True)
    keep = np.zeros(E, bool)
    keep[first_pos] = True
    grp_of_uniq = uniq_keys // U
    cnt = np.bincount(grp_of_uniq, minlength=B * MC)
    grp = row_idx * MC + layer_idx
    w = np.where(keep, 1.0 / np.maximum(cnt[grp], 1), 0.0).astype(np.float32)
    dest_all = (row_idx % BC) * MC + layer_idx      # core-local dest column

    # per (core, bucket): dest-sorted edge lists
    core_lists = []
    for c in range(NCORES):
        sel = (row_idx >= c * BC) & (row_idx < (c + 1) * BC)
        e_eff, e_dest, e_w = eff[sel], dest_all[sel], w[sel]
        per_bucket = []
        for r in range(2):
            bsel = (e_eff >= BUCKET_LO[r]) & (e_eff < (BUCKET_LO[1] if r == 0 else N))
            order = np.argsort(e_dest[bsel], kind="stable")
            per_bucket.append((
                (e_eff[bsel][order] - BUCKET_BASES[r]).astype(np.int32),
                e_dest[bsel][order].astype(np.int32),
                e_w[bsel][order],
            ))
        core_lists.append(per_bucket)

    # dense chunking: chunk = 128 consecutive dest-sorted edges; chunk count =
    # max over cores (shared compiled module), shorter cores pad (idx 0, w 0).
    nchk = []
    for r in range(2):
        mx = max(len(core_lists[c][r][0]) for c in range(NCORES))
        nchk.append((mx + 127) // 128)

    core_streams = []   # [core][bucket] -> (idx, dest, w) padded to nchk*128
    for c in range(NCORES):
        per_bucket = []
        for r in range(2):
            idx_rel, dests, ws = core_lists[c][r]
            cap = nchk[r] * 128
            s_idx = np.zeros(cap, np.int32)
            s_dst = np.full(cap, -1, np.int32)
            s_w = np.zeros(cap, np.float32)
            n = len(idx_rel)
            # spread edges evenly over the chunk grid (quantile alignment
            # across cores keeps per-chunk dest unions tight)
            bnd = np.round(np.arange(nchk[r] + 1) * n / nchk[r]).astype(np.int64)
            for k in range(nchk[r]):
                e0, e1 = bnd[k], bnd[k + 1]
                s_idx[k * 128:k * 128 + e1 - e0] = idx_rel[e0:e1]
                s_dst[k * 128:k * 128 + e1 - e0] = dests[e0:e1]
                s_w[k * 128:k * 128 + e1 - e0] = ws[e0:e1]
            per_bucket.append((s_idx, s_dst, s_w))
        core_streams.append(per_bucket)

    # gather pieces: runs of <= PIECE chunks
    pieces = []
    for r in range(2):
        bounds = list(range(0, nchk[r], PIECE)) + [nchk[r]]
        pieces.append([(bounds[i], bounds[i + 1]) for i in range(len(bounds) - 1)])

    # each gather piece must END on idx >= 0 (ucode pops trailing negatives):
    # swap a nonneg idx (pads are 0) into the last slot within the final chunk
    for c in range(NCORES):
        for r in range(2):
            s_idx, s_dst, s_w = core_streams[c][r]
            for (k0, k1) in pieces[r]:
                last = k1 * 128 - 1
                if s_idx[last] >= 0:
                    continue
                ch0 = (k1 - 1) * 128
                cand = np.nonzero(s_idx[ch0:last] >= 0)[0]
                assert len(cand), "whole final chunk of a piece is negative"
                j = ch0 + cand[0]
                for arr in (s_idx, s_dst, s_w):
                    arr[j], arr[last] = arr[last], arr[j]

    # per-chunk dest spans = union over cores of real dests
    spans = []          # (r, k, lo, hi)
    for r in range(2):
        for k in range(nchk[r]):
            lo, hi = DEST, 0
            for c in range(NCORES):
                d = core_streams[c][r][1][k * 128:(k + 1) * 128]
                d = d[d >= 0]
                if len(d):
                    lo = min(lo, int(d.min()))
                    hi = max(hi, int(d.max()) + 1)
            if hi <= lo:
                lo, hi = -1, -1
            spans.append((r, k, lo, hi))

    # segments: split spans at 256-column quarter-tile boundaries
    segs = []
    acol = 0
    for (r, k, lo, hi) in spans:
        if lo < 0:
            continue
        for t in range(4):
            b0, b1 = t * 256, (t + 1) * 256
            s0, s1 = max(lo, b0), min(hi, b1)
            if s1 > s0:
                segs.append(dict(bucket=r, chunk=k, tile=t, lo=s0, hi=s1,
                                 acol=acol + (s0 - lo)))
        acol += hi - lo
    aw = max(acol, 1)

    plan = dict(nchk=tuple(nchk), segs=segs, aw=aw,
                pieces=(tuple(pieces[0]), tuple(pieces[1])))

    percore = []
    span_acol = {}
    ac = 0
    for (r, k, lo, hi) in spans:
        span_acol[(r, k)] = (ac, lo)
        if lo >= 0:
            ac += hi - lo
    for c in range(NCORES):
        amat = np.zeros((P, aw), NPF16)
        widx = []
        for r in range(2):
            s_idx, s_dst, s_w = core_streams[c][r]
            assert s_idx.max(initial=0) <= 32767 and s_idx.min(initial=0) >= -32768
            widx.append(_wrap_idx16(s_idx))
            for k in range(nchk[r]):
                a0, lo = span_acol[(r, k)]
                if lo < 0:
                    continue
                sl = slice(k * 128, (k + 1) * 128)
                real = s_dst[sl] >= 0
                pp = np.nonzero(real)[0]
                amat[pp, a0 + s_dst[sl][pp] - lo] = s_w[sl][pp].astype(NPF16)
        sidx = np.zeros((P, 2), np.int32)
        sidx[:, 0] = nodes[c * BC: c * BC + 128]
        sidx[:, 1] = nodes[c * BC + 128: (c + 1) * BC]
        percore.append(dict(amat=amat, widx0=widx[0], widx1=widx[1],
                            widx=np.concatenate([widx[0], widx[1]], axis=1),
                            sidx=sidx))

    return plan, percore


def make_consts(inputs):
    """([128, 4*128] fp16 weights, [128, 2] f32 mu) shared across cores."""
    c = np.zeros((P, NSLOT * 128), NPF16)
    c[:, S_WK * 128:(S_WK + 1) * 128] = np.asarray(inputs["Wk"], np.float32)
    c[:, S_WQ * 128:(S_WQ + 1) * 128] = np.asarray(inputs["Wq"], np.float32)
    c[:, S_WVA * 128:(S_WVA + 1) * 128] = np.asarray(inputs["Wv_agg"], np.float32)
    c[:, S_WVF * 128:(S_WVF + 1) * 128] = np.asarray(inputs["Wv_ff"], np.float32)
    mu = np.asarray(inputs["mu_w"]).astype(np.float32).reshape(2, DOUT).T
    wva = np.asarray(inputs["Wv_agg"], np.float32)
    folded = np.stack([wva @ mu[:, 1], wva @ mu[:, 0]], axis=1)  # [w_num | w_smu]
    return c, np.ascontiguousarray(folded)


def make_big_table(inputs):
    """([N, 128] fp16 agg-only, [N, 256] fp16 agg|ff)."""
    agg16 = np.asarray(inputs["agg_table"], np.float32).astype(NPF16)
    big = np.empty((N, FW), NPF16)
    big[:, 0:DIN] = agg16
    big[:, DIN:FW] = np.asarray(inputs["ff_table"], np.float32)
    return agg16, big


# --------------------------------------------------------------------------
# device module
# --------------------------------------------------------------------------

def build_module(plan):
    nchk = plan["nchk"]
    aw = plan["aw"]
    segs = plan["segs"]

    nc = bacc.Bacc("TRN2", target_bir_lowering=False, debug=False,
                   num_devices=NCORES, num_swdge_queues=4)

    big_t = nc.dram_tensor("big", [N, FW], F16, kind="ExternalInput")
    consts = nc.dram_tensor("consts", [P, NSLOT * 128], F16, kind="ExternalInput")
    mu_d = nc.dram_tensor("mu", [P, 2], F32, kind="ExternalInput")
    amat = nc.dram_tensor("amat", [P, aw], F16, kind="ExternalInput")
    widx0 = nc.dram_tensor("widx0", [P, nchk[0] * 8], I16, kind="ExternalInput")
    widx1 = nc.dram_tensor("widx1", [P, nchk[1] * 8], I16, kind="ExternalInput")
    sidx = nc.dram_tensor("sidx", [P, 2], I32, kind="ExternalInput")
    out_t = nc.dram_tensor("out", [2, P, BC], F16, kind="ExternalOutput")

    # piece order on the gpsimd queue: self halves early (they gate the
    # highway front), then buckets interleaved so tile-0 chunks land first
    np0, np1 = len(plan["pieces"][0]), len(plan["pieces"][1])
    ei = []
    for i in range(max(np0, np1)):
        if i < np0:
            ei.append((0, *plan["pieces"][0][i]))
        if i < np1:
            ei.append((1, *plan["pieces"][1][i]))
    order = [('edge', *ei[0]), ('self', 0), ('self', 1)]
    order += [('edge', *e) for e in ei[1:]]

    # last chunk index per (tile) for stop flags
    last_per_tile = {}
    for i, s in enumerate(segs):
        last_per_tile[s["tile"]] = i

    with tile.TileContext(nc) as tc:
        with (
            nc.allow_low_precision(reason="fp16 pipeline validated vs 2e-2 tol"),
            tc.tile_pool(name="sb", bufs=1) as sb,
            tc.tile_pool(name="psA", bufs=4, space="PSUM") as psA,
            tc.tile_pool(name="ps", bufs=2, space="PSUM") as ps,
        ):
            def slot(k):
                return c_sb[:, k * 128:(k + 1) * 128]

            def syn(k):
                return syn_sb[:, k * 128:(k + 1) * 128]

            # ---- input DMAs (tiny index tensors first; A on the ACT ring)
            c_sb = sb.tile([P, NSLOT * 128], F16, tag="c_sb")
            a_sb = sb.tile([P, aw], F16, tag="a_sb")
            mu_sb = sb.tile([P, 2], F32, tag="mu_sb")
            w0_sb = sb.tile([P, nchk[0] * 8], I16, tag="w0_sb")
            w1_sb = sb.tile([P, nchk[1] * 8], I16, tag="w1_sb")
            si_sb = sb.tile([P, 2], I32, tag="si_sb")
            nc.sync.dma_start(out=w0_sb[:], in_=widx0[:, :])
            nc.sync.dma_start(out=w1_sb[:], in_=widx1[:, :])
            nc.sync.dma_start(out=si_sb[:], in_=sidx[:, :])
            nc.sync.dma_start(out=c_sb[:], in_=consts[:, :])
            nc.sync.dma_start(out=mu_sb[:], in_=mu_d[:, :])
            nc.scalar.dma_start(out=a_sb[:], in_=amat[:, :])

            # prime the single ACT table set (exp+ln) at t=0
            warm = sb.tile([P, 1], F32, tag="warm")
            nc.vector.memset(warm[:], 1.0)
            warm2 = sb.tile([P, 2], F32, tag="warm2")
            nc.scalar.activation(warm2[:, 0:1], warm[:], AF.Ln)
            nc.scalar.activation(warm2[:, 1:2], warm[:], AF.Exp)

            # synthesized constants (fp16)
            syn_sb = sb.tile([P, 5 * 128], F16, tag="syn_sb")
            nc.vector.memset(syn_sb[:, S_ONES * 128:(S_ONES + 1) * 128], 1.0)
            nc.vector.memset(syn_sb[:, S_ONESC * 128:(S_ONESC + 1) * 128], 1.0 / DOUT)
            # mu_d col0 = w_num (neighbor vector), col1 = w_smu (self vector)
            nc.vector.tensor_copy(syn_sb[:, S_MUHI * 128:(S_MUHI + 1) * 128],
                                  mu_sb[:, 0:1].to_broadcast((P, 128)))
            nc.vector.tensor_copy(syn_sb[:, S_MULO * 128:(S_MULO + 1) * 128],
                                  mu_sb[:, 1:2].to_broadcast((P, 128)))
            make_identity(nc, syn_sb[:, S_ID * 128:(S_ID + 1) * 128])

            # aggregation psums zeroed early (before any dependent DVE work
            # queues ahead of them in the in-order DVE stream)
            pagg = [psA.tile([P, 512], F32, tag="pagg", name=f"pagg{i}")
                    for i in range(2)]
            nc.vector.memset(pagg[0][:], 0.0)
            nc.vector.memset(pagg[1][:], 0.0)

            # ---- dummy gather: loads the mlp ucode library at t=0
            dum_i = sb.tile([P, 8], I16, tag="dum_i")
            nc.gpsimd.memset(dum_i[:], 0)
            dum_o = sb.tile([P, 1, 128], F16, tag="dum_o")
            nc.gpsimd.dma_gather(dum_o[:], big_t[:, 0:128], dum_i[:], 128, 128, 128,
                                 elem_step=FW, queue_num=1)

            # ---- gathers (edge pieces + self indirects) in `order`
            g0 = sb.tile([P, nchk[0], 128], F16, tag="g0")
            g1 = sb.tile([P, nchk[1], 128], F16, tag="g1")
            sr = sb.tile([P, 2, FW], F16, tag="sr")     # [part=b%128, h, agg|ff]
            gtiles = (g0, g1)
            tiles_w = ((g0, w0_sb), (g1, w1_sb))
            gq = 0
            for item in order:
                if item[0] == 'self':
                    h = item[1]
                    nc.gpsimd.indirect_dma_start(
                        out=sr[:, h, :], out_offset=None,
                        in_=big_t[:, :],
                        in_offset=bass.IndirectOffsetOnAxis(ap=si_sb[:, h:h + 1], axis=0))
                else:
                    _, r, k0, k1 = item
                    gt, wt = tiles_w[r]
                    nc.gpsimd.dma_gather(
                        gt[:, k0:k1, :], big_t[BUCKET_BASES[r]:, 0:128],
                        wt[:, k0 * 8:k1 * 8],
                        (k1 - k0) * 128, (k1 - k0) * 128, 128,
                        elem_step=FW, queue_num=gq % 4)
                    gq += 1

            # ---- pair transposes: pair_T [128 feat, 512] =
            #      [aggT_h0 | aggT_h1 | ffT_h0 | ffT_h1]
            pair_T = sb.tile([P, 512], F16, tag="pair_T")
            for h in range(2):
                tpa = ps.tile([P, 128], F16, tag="ps_rot", name=f"tpa{h}")
                nc.tensor.transpose(tpa[:], sr[:, h, 0:128], syn(S_ID))
                nc.scalar.copy(pair_T[:, h * 128:(h + 1) * 128], tpa[:])
                tpf = ps.tile([P, 128], F16, tag="ps_rot", name=f"tpf{h}")
                nc.tensor.transpose(tpf[:], sr[:, h, 128:256], syn(S_ID))
                nc.scalar.copy(pair_T[:, 256 + h * 128: 256 + (h + 1) * 128], tpf[:])

            # ---- early dense from pair_T: K/Q (both branches), vf, self acts
            kt_ps = ps.tile([P, 512], F32, tag="ps_kq", name="kt_ps", bufs=1)
            nc.tensor.matmul(out=kt_ps[:], lhsT=slot(S_WK), rhs=pair_T[:],
                             start=True, stop=True)
            kt = sb.tile([P, 512], F16, tag="kt")
            nc.scalar.copy(kt[:], kt_ps[:])
            qt_ps = ps.tile([P, 512], F32, tag="ps_kq", name="qt_ps", bufs=1)
            nc.tensor.matmul(out=qt_ps[:], lhsT=slot(S_WQ), rhs=pair_T[:],
                             start=True, stop=True)
            qt = sb.tile([P, 512], F16, tag="qt")
            nc.scalar.copy(qt[:], qt_ps[:])
            vfs_ps = ps.tile([P, 512], F32, tag="ps_big", name="vfs_ps", bufs=1)
            nc.tensor.matmul(out=vfs_ps[:, 0:256], lhsT=slot(S_WVF),
                             rhs=pair_T[:, 256:512], start=True, stop=True,
                             skip_group_check=True)
            nc.tensor.matmul(out=vfs_ps[:, 256:512], lhsT=slot(S_WVA),
                             rhs=pair_T[:, 0:256], start=True, stop=True,
                             skip_group_check=True)
            vf = sb.tile([P, 256], F16, tag="vf")
            nc.scalar.copy(vf[:], vfs_ps[:, 0:256])
            acts = sb.tile([P, 256], F16, tag="acts")   # self branch acts
            nc.vector.tensor_copy(acts[:], vfs_ps[:, 256:512])
            sqs = sb.tile([P, 256], F16, tag="sqs")
            nc.vector.tensor_mul(sqs[:], acts[:], acts[:])
            self_half = sb.tile([P, 256], F16, tag="self_half")
            nc.scalar.mul(self_half[:], acts[:], 0.5)
            vf01 = sb.tile([P, 256], F16, tag="vf01")
            nc.scalar.mul(vf01[:], vf[:], 1.0 - RES_RATE)

            # highway front: dif_i = colsum(K_i * (Q_agg - Q_ff)) / DOUT
            qd = sb.tile([P, 256], F16, tag="qd")
            nc.vector.tensor_sub(qd[:], qt[:, 0:256], qt[:, 256:512])
            pd = sb.tile([P, 512], F16, tag="pd")
            nc.vector.tensor_mul(pd[:, 0:256], kt[:, 0:256], qd[:])
            nc.vector.tensor_mul(pd[:, 256:512], kt[:, 256:512], qd[:])
            dif_ps = ps.tile([P, 512], F32, tag="ps_big", name="dif_ps", bufs=1)
            nc.tensor.matmul(out=dif_ps[:], lhsT=syn(S_ONESC), rhs=pd[:],
                             start=True, stop=True)
            eneg = sb.tile([P, 512], F16, tag="eneg")
            nc.scalar.activation(eneg[:], dif_ps[:], AF.Exp, scale=-1.0)
            wden = sb.tile([P, 512], F16, tag="wden")
            nc.vector.tensor_scalar_add(wden[:], eneg[:], 1.0)
            wgt = sb.tile([P, 512], F16, tag="wgt")      # [waa 256 | wfa 256]
            nc.vector.reciprocal(wgt[:], wden[:])

            # ---- per-half chains
            nraw = sb.tile([P, 1024], F16, tag="nraw")
            actn = sb.tile([P, 1024], F16, tag="actn")
            sqn = sb.tile([P, 1024], F16, tag="sqn")
            logit = sb.tile([P, 1024], F16, tag="logit")
            esm = sb.tile([P, 1024], F16, tag="esm")
            tmul = sb.tile([P, 1024], F16, tag="tmul")
            tsum = sb.tile([P, 256], F32, tag="tsum")
            ssum = sb.tile([P, 256], F32, tag="ssum")
            rsum = sb.tile([P, 256], F32, tag="rsum")
            nsum = sb.tile([P, 256], F16, tag="nsum")
            vmid = sb.tile([P, 256], F16, tag="vmid")
            lden = sb.tile([P, 1024], F16, tag="lden")
            rden = sb.tile([P, 1024], F16, tag="rden")
            pre = sb.tile([P, 512], F16, tag="pre")     # [agg_h0|agg_h1|ff_h0|ff_h1]
            out_sb = sb.tile([P, 512], F16, tag="out_sb")

            rep4 = lambda apx: apx[:, :, None].to_broadcast((P, 128, MC))

            for h in range(2):
                hs = slice(h * 512, (h + 1) * 512)
                for i, s in enumerate(segs):
                    if s["tile"] != h:
                        continue
                    nc.tensor.matmul(
                        out=pagg[h][:, s["lo"] - h * 512: s["hi"] - h * 512],
                        lhsT=gtiles[s["bucket"]][:, s["chunk"], :],
                        rhs=a_sb[:, s["acol"]: s["acol"] + s["hi"] - s["lo"]],
                        start=False, stop=(last_per_tile[h] == i),
                        skip_group_check=True)
                # PSUM -> SBUF fp16 copy, split ACT || DVE
                nc.scalar.copy(nraw[:, h * 512:h * 512 + 256], pagg[h][:, 0:256])
                nc.vector.tensor_copy(nraw[:, h * 512 + 256:(h + 1) * 512],
                                      pagg[h][:, 256:512])
                actn_ps = psA.tile([P, 512], F32, tag="pagg", name=f"actn_ps{h}")
                nc.tensor.matmul(out=actn_ps[:], lhsT=slot(S_WVA), rhs=nraw[:, hs],
                                 start=True, stop=True)
                nc.scalar.copy(actn[:, h * 512:h * 512 + 256], actn_ps[:, 0:256])
                nc.vector.tensor_copy(actn[:, h * 512 + 256:(h + 1) * 512],
                                      actn_ps[:, 256:512])
                nc.vector.tensor_mul(sqn[:, hs], actn[:, hs], actn[:, hs])
                # stats matmuls with self fold-in (0-stride broadcast rhs):
                # den2 = ones@sqn + ones@rep4(sqs_h); num = muhi@actn + mulo@rep4(acts_h)
                den2_ps = psA.tile([P, 512], F32, tag="pagg", name=f"den2_ps{h}")
                nc.tensor.matmul(out=den2_ps[:], lhsT=syn(S_ONES), rhs=sqn[:, hs],
                                 start=True, stop=False, skip_group_check=True)
                nc.tensor.matmul(out=den2_ps[:].rearrange("p (b m) -> p b m", m=MC),
                                 lhsT=syn(S_ONES),
                                 rhs=rep4(sqs[:, h * 128:(h + 1) * 128]),
                                 start=False, stop=True, skip_group_check=True)
                num_ps = psA.tile([P, 512], F32, tag="pagg", name=f"num_ps{h}")
                nc.tensor.matmul(out=num_ps[:], lhsT=syn(S_MUHI), rhs=actn[:, hs],
                                 start=True, stop=False, skip_group_check=True)
                nc.tensor.matmul(out=num_ps[:].rearrange("p (b m) -> p b m", m=MC),
                                 lhsT=syn(S_MULO),
                                 rhs=rep4(acts[:, h * 128:(h + 1) * 128]),
                                 start=False, stop=True, skip_group_check=True)
                # rden = exp(-0.5 ln(den2)); logit = num * rden
                nc.scalar.activation(lden[:, hs], den2_ps[:], AF.Ln)
                nc.scalar.activation(rden[:, hs], lden[:, hs], AF.Exp, scale=-0.5)
                nc.vector.tensor_mul(logit[:, hs], num_ps[:], rden[:, hs])
                nc.scalar.activation(esm[:, hs], logit[:, hs], AF.Exp)
                nc.vector.tensor_mul(tmul[:, hs], esm[:, hs], actn[:, hs])
                ts_h = slice(h * 128, (h + 1) * 128)
                nc.vector.reduce_sum(
                    out=tsum[:, ts_h],
                    in_=tmul[:, hs].rearrange("p (b m) -> p b m", m=MC),
                    axis=mybir.AxisListType.X)
                nc.vector.reduce_sum(
                    out=ssum[:, ts_h],
                    in_=esm[:, hs].rearrange("p (b m) -> p b m", m=MC),
                    axis=mybir.AxisListType.X)
                nc.vector.reciprocal(rsum[:, ts_h], ssum[:, ts_h])
                nc.vector.tensor_mul(nsum[:, ts_h], tsum[:, ts_h], rsum[:, ts_h])
                nc.vector.scalar_tensor_tensor(
                    out=vmid[:, ts_h], in0=nsum[:, ts_h], scalar=0.5,
                    in1=self_half[:, ts_h], op0=OP.mult, op1=OP.add)

                # residual mix + ELU per half:
                #   pre_agg = 0.9*vmid + 0.1*vf + 0.1*waa*dd
                #   pre_ff  = vf + 0.1*wfa*dd
                dd_h = sb.tile([P, 128], F16, tag=f"dd{h}", name=f"dd{h}")
                nc.vector.tensor_sub(dd_h[:], vmid[:, ts_h], vf[:, ts_h])
                base_h = sb.tile([P, 128], F16, tag=f"base{h}", name=f"base{h}")
                nc.vector.scalar_tensor_tensor(
                    out=base_h[:], in0=vmid[:, ts_h], scalar=RES_RATE,
                    in1=vf01[:, ts_h], op0=OP.mult, op1=OP.add)
                for o, b9 in enumerate((base_h[:], vf[:, ts_h])):
                    nw = sb.tile([P, 128], F16, tag=f"nw{h}{o}", name=f"nw{h}{o}")
                    nc.vector.tensor_mul(nw[:], wgt[:, o * 256 + h * 128:
                                                    o * 256 + (h + 1) * 128], dd_h[:])
                    nc.vector.scalar_tensor_tensor(
                        out=pre[:, o * 256 + h * 128: o * 256 + (h + 1) * 128],
                        in0=nw[:], scalar=1.0 - RES_RATE, in1=b9,
                        op0=OP.mult, op1=OP.add)
                # ELU: relu(x) + min(exp(x), 1) - 1
                for o in range(2):
                    sl = slice(o * 256 + h * 128, o * 256 + (h + 1) * 128)
                    ep = sb.tile([P, 128], F16, tag=f"ep{h}{o}", name=f"ep{h}{o}")
                    nc.scalar.activation(ep[:], pre[:, sl], AF.Exp)
                    rp = sb.tile([P, 128], F16, tag=f"rp{h}{o}", name=f"rp{h}{o}")
                    nc.vector.tensor_scalar_max(rp[:], pre[:, sl], 0.0)
                    em = sb.tile([P, 128], F16, tag=f"em{h}{o}", name=f"em{h}{o}")
                    nc.vector.tensor_scalar(out=em[:], in0=ep[:], scalar1=1.0,
                                            scalar2=-1.0, op0=OP.min, op1=OP.add)
                    nc.vector.tensor_add(out_sb[:, sl], em[:], rp[:])
                # per-half output DMA: out[c, :, h*128:(h+1)*128]
                nc.sync.dma_start(
                    out=out_t[:, :, h * 128:(h + 1) * 128].rearrange("c d b -> d c b"),
                    in_=out_sb[:].rearrange("p (c b) -> p c b", b=256)[:, :, h * 128:(h + 1) * 128])

    nc.compile()
    return nc


# --------------------------------------------------------------------------
# numpy simulation of the device pipeline (validates preprocessing + math)
# --------------------------------------------------------------------------

def numpy_simulate(inputs, plan, percore):
    big = make_big_table(inputs)[1].astype(np.float32)
    cmat, mu2 = make_consts(inputs)
    cmat = cmat.astype(np.float32)
    outs_a, outs_f = [], []
    for c in range(NCORES):
        pc = percore[c]
        def unwrap(widx, nchunks):
            w16 = widx[:16, :]
            return w16.T.reshape(-1).astype(np.int32)[: nchunks * 128]
        g = []
        for r, widx in enumerate((pc["widx0"], pc["widx1"])):
            idx = unwrap(widx, plan["nchk"][r]) + BUCKET_BASES[r]
            g.append(big[idx, 0:128].reshape(plan["nchk"][r], 128, 128).transpose(1, 0, 2))
        srn = big[pc["sidx"].T.reshape(-1)]          # [256, 256] node-major
        pair_T = np.concatenate([srn[:, 0:128].T, srn[:, 128:256].T], axis=1)
        pagg = np.zeros((4, P, 256), np.float32)
        for s in plan["segs"]:
            G = g[s["bucket"]][:, s["chunk"], :]
            A = pc["amat"].astype(np.float32)[:, s["acol"]: s["acol"] + s["hi"] - s["lo"]]
            pagg[s["tile"]][:, s["lo"] - s["tile"] * 256: s["hi"] - s["tile"] * 256] += G.T @ A
        neigh_rawT = np.concatenate(list(pagg), axis=1)
        Wva = cmat[:, S_WVA * 128:(S_WVA + 1) * 128]
        Wvf = cmat[:, S_WVF * 128:(S_WVF + 1) * 128]
        Wk = cmat[:, S_WK * 128:(S_WK + 1) * 128]
        Wq = cmat[:, S_WQ * 128:(S_WQ + 1) * 128]
        actn = Wva.T @ neigh_rawT                     # [128, 1024]
        acts = Wva.T @ pair_T[:, 0:256]               # [128, 256] self
        vf = Wvf.T @ pair_T[:, 256:512]
        kt = Wk.T @ pair_T
        qt = Wq.T @ pair_T
        n2 = (actn * actn).sum(0)
        s2 = (acts * acts).sum(0)
        w_num, w_smu = mu2[:, 0:1], mu2[:, 1:2]
        nmu = (w_num * neigh_rawT).sum(0)
        smu = (w_smu * pair_T[:, 0:256]).sum(0)
        den2 = n2 + np.repeat(s2, MC)
        numv = nmu + np.repeat(smu, MC)
        logit = numv / np.sqrt(den2)
        e = np.exp(logit).reshape(BC, MC)
        coef = e / e.sum(1, keepdims=True)
        neighT = actn.reshape(P, BC, MC)
        nsum = (neighT * coef[None]).sum(-1)
        vmid = 0.5 * (acts + nsum)
        saa = (kt[:, 0:256] * qt[:, 0:256]).sum(0) / DOUT
        saf = (kt[:, 0:256] * qt[:, 256:512]).sum(0) / DOUT
        sfa = (kt[:, 256:512] * qt[:, 0:256]).sum(0) / DOUT
        sff = (kt[:, 256:512] * qt[:, 256:512]).sum(0) / DOUT
        waa = 1.0 / (1.0 + np.exp(-(saa - saf)))
        wfa = 1.0 / (1.0 + np.exp(-(sfa - sff)))
        dd = vmid - vf
        new0 = vf + waa[None] * dd
        new1 = vf + wfa[None] * dd
        pre0 = RES_RATE * vmid + (1 - RES_RATE) * new0
        pre1 = RES_RATE * vf + (1 - RES_RATE) * new1
        elu = lambda x: np.where(x > 0, x, np.exp(np.minimum(x, 0)) - 1)
        outs_a.append(elu(pre0).T)
        outs_f.append(elu(pre1).T)
    return np.concatenate(outs_a, 0), np.concatenate(outs_f, 0)


# --------------------------------------------------------------------------
# public entry point
# --------------------------------------------------------------------------

_module_cache = {}
_last_results = None


def _plan_signature(plan):
    return (plan["nchk"], plan["aw"], plan["pieces"],
            tuple((s["bucket"], s["chunk"], s["tile"], s["lo"], s["hi"], s["acol"])
                  for s in plan["segs"]))


def kernel(**inputs):
    plan, percore = preprocess(inputs)
    sig = _plan_signature(plan)
    if sig not in _module_cache:
        _module_cache[sig] = build_module(plan)
    nc = _module_cache[sig]

    cmat, mu2 = make_consts(inputs)
    agg16, big = make_big_table(inputs)
    in_maps = []
    for c in range(NCORES):
        pc = percore[c]
        in_maps.append({
            "agg16": agg16,
            "big": big,
            "consts": cmat,
            "mu": mu2,
            "amat": pc["amat"],
            "widx": pc["widx"],
            "sidx": pc["sidx"],
        })
    res = run_bass_kernel_spmd(nc, in_maps, core_ids=list(range(NCORES)))
    global _last_results
    _last_results = res
    agg_out = np.concatenate(
        [res.results[c]["out"][0].astype(np.float32).T for c in range(NCORES)], axis=0)
    ff_out = np.concatenate(
        [res.results[c]["out"][1].astype(np.float32).T for c in range(NCORES)], axis=0)
    return agg_out, ff_out
